# revision 21
# baseline (speedup 1.0000x reference)
"""MoE (top-4 of 32 experts) Trainium2 kernel, data-parallel over 8 NeuronCores.

Problem shapes: x[4096,512] f32, expert_sel[32,512] f32, w1[32,512,128] f32,
w2[32,128,512] f32 -> y[4096,512] f32.

Strategy: shard the 4096 tokens 512-per-core (no collectives). Each core:
  - scores = x @ expert_sel.T in fp32 on PE (routing must match the
    reference's fp32 ranking exactly, so this stays fp32)
  - sigmoid on ACT; top-4 threshold per token via the DVE Max8 instruction;
    gate = sigmoid(score) * (score >= 4th_max), in bf16
  - gate rows are transposed to [E, T] with DVE 32x32 stream transposes,
    bounced through DRAM, and DMA-broadcast-read back as [128, T] per group
    of experts (SBUF APs cannot broadcast the partition dim; DRAM APs can)
  - dense expert MLPs in bf16, phase-split: all 32 experts' L1
    (h = relu(x @ w1[e]), gated to hg on DVE) first, then L2 in dc-major
    order (for each 128-wide output chunk: 32 accumulating matmuls into one
    PSUM bank) so each output chunk is evicted and DMA'd while the next
    chunk computes -- the write-out tail shrinks to one chunk.
  - PE warm-up starts immediately (DVE memset, not gpsimd) so the HAM clock
    gate reaches 8/8 during the input DMA stream.
  - all heavy DMAs are host-packed contiguous 2D transfers, enqueued on the
    SP HWDGE in need order.
"""

import os
import numpy as np
import ml_dtypes

N, D, E, H, K = 4096, 512, 32, 128, 4
NCORES = 8
TPC = N // NCORES  # tokens per core = 512
P = 128
DC = D // P        # 4 contraction chunks of d
TT = TPC // P      # 4 token tiles of 128
G = 4              # experts per DMA group
NG = E // G
BF16 = ml_dtypes.bfloat16

_CACHE = {}


def _split_waits_json(bir_bytes, max_waits=1):
    """The walrus build in this container encodes at most one sync-wait per
    instruction; Tile emits several on some. Split excess waits onto
    preceding same-engine NoOps (identical semantics: program order on the
    engine)."""
    import orjson

    bir = orjson.loads(bir_bytes)
    nid = [0]

    def fix_block(instructions):
        out = []
        for ins in instructions:
            si = ins.get("sync_info")
            waits = (si or {}).get("on_wait") or []
            if len(waits) > max_waits:
                chunks = [
                    waits[i : i + max_waits] for i in range(0, len(waits), max_waits)
                ]
                for ch in chunks[:-1]:
                    nid[0] += 1
                    out.append(
                        {
                            "opcode": "NoOp",
                            "engine": ins["engine"],
                            "ins": [],
                            "outs": [],
                            "name": f"I-ws-{nid[0]}",
                            "debug": ins.get("debug", 0),
                            "sync_info": {"on_wait": ch, "on_update": []},
                        }
                    )
                si["on_wait"] = chunks[-1]
            out.append(ins)
        return out

    def walk(o):
        if isinstance(o, dict):
            for k, v in o.items():
                if k == "instructions" and isinstance(v, list):
                    o[k] = fix_block(v)
                else:
                    walk(v)
        elif isinstance(o, list):
            for v in o:
                walk(v)

    walk(bir)
    return orjson.dumps(bir)


LDW_OPT = False


def _patch_compile():
    if _CACHE.get("patched"):
        return
    import concourse.bass2jax as bass2jax
    import concourse.bass_utils as bass_utils
    from concourse.bass_utils import compile_bir_kernel as _orig

    if LDW_OPT and not _CACHE.get("ldw_patched"):
        _orig_run = bass_utils.run_command

        def run2(cmd, **kw):
            cmd = [
                "--enable-ldw-opt=true" if c == "--enable-ldw-opt=false" else c
                for c in cmd
            ]
            return _orig_run(cmd, **kw)

        bass_utils.run_command = run2
        _CACHE["ldw_patched"] = True

    def patched(bir_json, tmpdir, neff_name="file.neff"):
        return _orig(_split_waits_json(bir_json), tmpdir, neff_name=neff_name)

    bass2jax.compile_bir_kernel = patched
    _CACHE["patched"] = True


def _patch_tile_tail():
    # Tile's kernel epilogue is drain + barrier + sem-clears + barrier. The
    # second barrier only orders the clears vs engine program-end; NRT
    # already requires every engine's program to finish before the NEFF can
    # run again, so it is dead time (~2-3us). Drop it.
    if _CACHE.get("tail_patched"):
        return
    from concourse.tile import TileContext
    from concourse.vector_clock import ScopedClock

    def _dab(self, tick_clock, wait_clock):
        drain_inst = self.nc.sync.drain()
        wait_clock.add_sem_waits(
            drain_inst.ins, ScopedClock({None: tick_clock.global_clock})
        )
        self.nc.all_engine_barrier()
        popped = self.nc._tile_sem_poison_stack.pop()
        assert popped is self._sem_poison
        self.nc.clear_and_free_semaphores(list(self.sems.allocated().values()))

    TileContext._drain_and_barrier = _dab
    _CACHE["tail_patched"] = True


def _build_nc():
    import concourse.bass as bass
    import concourse.mybir as mybir
    from concourse.tile import TileContext

    _patch_tile_tail()

    dt = mybir.dt
    Alu = mybir.AluOpType
    Act = mybir.ActivationFunctionType

    nc = bass.Bass()

    xT_d = nc.dram_tensor("xT", [P, DC * TPC], dt.float32, kind="ExternalInput")
    xbT_d = nc.dram_tensor("xbT", [P, DC * TPC], dt.bfloat16, kind="ExternalInput")
    selT_d = nc.dram_tensor("selT", [P, DC * E], dt.float32, kind="ExternalInput")
    # w1 packed per group of G experts: [NG, P(d-part), G * DC * H]
    w1_d = nc.dram_tensor("w1p", [NG, P, G * DC * H], dt.bfloat16, kind="ExternalInput")
    # w2 packed per group of G experts: [NG, P(h), G * D]
    w2_d = nc.dram_tensor("w2p", [NG, P, G * D], dt.bfloat16, kind="ExternalInput")
    yT_d = nc.dram_tensor("yT", [DC, P, TPC], dt.float32, kind="ExternalOutput")

    with TileContext(nc) as tc:
        with (
            tc.tile_pool(name="singles", bufs=1) as singles,
            tc.tile_pool(name="dram", bufs=1, space="DRAM") as drampool,
            tc.tile_pool(name="gpool", bufs=8) as gpool,
            tc.tile_pool(name="hrpool", bufs=16) as hrpool,
            tc.tile_pool(name="ph", bufs=5, space="PSUM") as phpool,
            tc.tile_pool(name="py", bufs=2, space="PSUM") as pypool,
        ):
            xf = [
                singles.tile([P, DC * P], dt.float32, name=f"xf{tt}")
                for tt in range(TT)
            ]
            xb = singles.tile([P, DC * TPC], dt.bfloat16)
            sel = singles.tile([P, DC * E], dt.float32)
            sig = singles.tile([P, TT * E], dt.float32)
            m8 = singles.tile([P, TT * 8], dt.float32)
            gate = singles.tile([P, TT * E], dt.bfloat16)
            gTb = singles.tile([32, TPC], dt.bfloat16)
            w1s = singles.tile([P, E * DC * H], dt.bfloat16)   # 32KB/part
            w2s = singles.tile([P, E * D], dt.bfloat16)        # 32KB/part
            hgs = singles.tile([P, E * TPC], dt.bfloat16)      # 32KB/part
            y_sb = singles.tile([P, DC * TPC], dt.float32)
            gTd = drampool.tile([E, TPC], dt.bfloat16)

            ggrp = {}
            hrs = {}
            TH2 = TPC // 2

            def scores_tile(tt):
                psc = phpool.tile([P, E], dt.float32, tag="ph", name=f"psc{tt}")
                for dc in range(DC):
                    nc.tensor.matmul(
                        psc[:],
                        xf[tt][:, dc * P : (dc + 1) * P],
                        sel[:, dc * E : (dc + 1) * E],
                        start=(dc == 0),
                        stop=(dc == DC - 1),
                    )
                sl = slice(tt * E, (tt + 1) * E)
                nc.scalar.activation(sig[:, sl], psc[:], Act.Sigmoid)
                # top-8 of the sigmoid (same ranking as raw scores)
                nc.vector.max(m8[:, tt * 8 : (tt + 1) * 8], sig[:, sl])
                # gate = (sig >= 4th max) * sig, in bf16
                nc.vector.scalar_tensor_tensor(
                    gate[:, sl],
                    sig[:, sl],
                    m8[:, tt * 8 + 3 : tt * 8 + 4],
                    sig[:, sl],
                    op0=Alu.is_ge,
                    op1=Alu.mult,
                )
                # transpose this token-tile of the gate to [E, T] and bounce
                # it through DRAM immediately (per-tile, to shorten the chain)
                for pb in range(TT):
                    nc.vector.transpose(
                        gTb[0:32, tt * P + pb * 32 : tt * P + (pb + 1) * 32],
                        gate[pb * 32 : (pb + 1) * 32, sl],
                    )
                nc.gpsimd.dma_start(
                    gTd[:, tt * P : (tt + 1) * P], gTb[0:32, tt * P : (tt + 1) * P]
                )

            def g_group(gi):
                g = gpool.tile([P, G * TPC], dt.bfloat16, tag="g", name=f"g{gi}")
                base = gTd[gi * G : (gi + 1) * G, :]
                src = bass.AP(base.tensor, base.offset, [[0, P]] + list(base.ap))
                nc.sync.dma_start(g[:].rearrange("p (e t) -> p e t", e=G), src)
                ggrp[gi] = g

            def l1_mm(e):
                wbase = e * DC * H
                ph = phpool.tile([P, TPC], dt.float32, tag="ph", name=f"ph{e}")
                for dc in range(DC):
                    nc.tensor.matmul(
                        ph[:],
                        w1s[:, wbase + dc * H : wbase + (dc + 1) * H],
                        xb[:, dc * TPC : (dc + 1) * TPC],
                        start=(dc == 0),
                        stop=(dc == DC - 1),
                    )
                hr = hrpool.tile([P, TPC], dt.bfloat16, tag="hr", name=f"hr{e}")
                nc.scalar.activation(hr[:], ph[:], Act.Relu)
                hrs[e] = hr

            def l1_mul(e):
                gi, ei = divmod(e, G)
                hr = hrs.pop(e)
                nc.vector.tensor_mul(
                    hgs[:, e * TPC : (e + 1) * TPC],
                    hr[:],
                    ggrp[gi][:, ei * TPC : (ei + 1) * TPC],
                )

            def l2_chunk(dtile):
                py = pypool.tile([P, TPC], dt.float32, tag="py", name=f"py{dtile}")
                for e in range(E):
                    nc.tensor.matmul(
                        py[:],
                        w2s[:, e * D + dtile * P : e * D + (dtile + 1) * P],
                        hgs[:, e * TPC : (e + 1) * TPC],
                        start=(e == 0),
                        stop=(e == E - 1),
                    )
                sl = slice(dtile * TPC, (dtile + 1) * TPC)
                if dtile % 2 == 0:
                    nc.vector.tensor_copy(y_sb[:, sl], py[:])
                else:
                    nc.scalar.activation(y_sb[:, sl], py[:], Act.Copy)
                nc.sync.dma_start(yT_d[dtile], y_sb[:, sl])

            # SP HWDGE enqueue in need order. The routing inputs go first:
            # the gate path is the long pole.
            src = xT_d[:].rearrange("p (c t) -> p c t", c=DC)
            nc.sync.dma_start(
                xf[0][:].rearrange("p (c t) -> p c t", c=DC), src[:, :, :P]
            )
            nc.sync.dma_start(sel[:], selT_d[:])
            for tt in range(1, TT):
                nc.sync.dma_start(
                    xf[tt][:].rearrange("p (c t) -> p c t", c=DC),
                    src[:, :, tt * P : (tt + 1) * P],
                )
            nc.sync.dma_start(xb[:], xbT_d[:])
            for gi in range(NG):
                nc.sync.dma_start(
                    w1s[:, gi * G * DC * H : (gi + 1) * G * DC * H], w1_d[gi]
                )
            # PE keep-warm while xf0/sel stream in (no input deps), and
            # between score tiles so the clock ramp isn't reset by the gaps
            # while the xf tiles trickle in.
            junk = singles.tile([P, 256], dt.bfloat16)
            nc.vector.memset(junk[:], 1.0)
            pj = phpool.tile([P, 256], dt.float32, tag="ph", name="pjunk")

            def keep_warm(n):
                for _ in range(n):
                    nc.tensor.matmul(
                        pj[:], junk[:, :P], junk[:], start=True, stop=True
                    )

            keep_warm(8)
            for tt in range(TT):
                scores_tile(tt)
                keep_warm(3)
            keep_warm(4)
            for gi in range(NG):
                g_group(gi)
            for gi in range(NG):
                nc.sync.dma_start(w2s[:, gi * G * D : (gi + 1) * G * D], w2_d[gi])
            for e in range(E):
                l1_mm(e)
                l1_mul(e)
            for dtile in range(DC):
                l2_chunk(dtile)

    return nc


def _get_nc():
    if "nc" not in _CACHE:
        _CACHE["nc"] = _build_nc()
    return _CACHE["nc"]


def _pack_inputs(x, expert_sel, w1, w2):
    x = np.asarray(x, dtype=np.float32)
    expert_sel = np.asarray(expert_sel, dtype=np.float32)
    w1 = np.asarray(w1, dtype=np.float32)
    w2 = np.asarray(w2, dtype=np.float32)

    # selT: [p, dc*E + e] = expert_sel[e, dc*P + p]
    selT = np.ascontiguousarray(
        expert_sel.T.reshape(DC, P, E).transpose(1, 0, 2)
    ).reshape(P, DC * E)
    # w1 per expert: [p(d-part), dc*H + h], grouped by G experts
    w1p = (
        w1.astype(BF16).reshape(E, DC, P, H).transpose(0, 2, 1, 3).reshape(E, P, DC * H)
    )
    w1p = np.ascontiguousarray(
        w1p.reshape(NG, G, P, DC * H).transpose(0, 2, 1, 3)
    ).reshape(NG, P, G * DC * H)
    # w2 per expert: [p(h), d], grouped by G experts
    w2p = np.ascontiguousarray(
        w2.astype(BF16).reshape(NG, G, P, D).transpose(0, 2, 1, 3)
    ).reshape(NG, P, G * D)

    in_maps = []
    for c in range(NCORES):
        xc = x[c * TPC : (c + 1) * TPC]
        # xT: [p, dc*TPC + t] = x[t, dc*P + p]
        xT = np.ascontiguousarray(
            xc.T.reshape(DC, P, TPC).transpose(1, 0, 2)
        ).reshape(P, DC * TPC)
        in_maps.append(
            {"xT": xT, "xbT": xT.astype(BF16), "selT": selT, "w1p": w1p, "w2p": w2p}
        )
    return in_maps


def _run(x, expert_sel, w1, w2, trace=False, tmpdir=None):
    _patch_compile()
    from concourse.bass_utils import run_bass_kernel_spmd

    if trace:
        _install_ntff_hook()

    nc = _get_nc()
    in_maps = _pack_inputs(x, expert_sel, w1, w2)
    res = run_bass_kernel_spmd(
        nc, in_maps, list(range(NCORES)), trace=trace, tmpdir=tmpdir
    )
    y = np.empty((N, D), dtype=np.float32)
    for c in range(NCORES):
        yT = np.asarray(res.results[c]["yT"], dtype=np.float32)
        y[c * TPC : (c + 1) * TPC] = yT.reshape(D, TPC).T
    return y, res


def _install_ntff_hook():
    """Register the NTFF profiling hook (the container's antenv stub lacks
    axon_hooks; replicate trn_boot's ctypes hook). Also stub the artifact
    upload, which needs cloud storage not present here."""
    if _CACHE.get("ntff"):
        return
    import sys, types, ctypes, contextlib
    import antenv  # noqa: F401
    import concourse.bass_utils as bass_utils

    bass_utils.upload_artifacts = lambda d: f"file://{d}"

    mod = types.ModuleType("antenv.axon_hooks")
    store = [None]
    mod.set_axon_ntff_profile_hook = lambda h: store.__setitem__(0, h)
    mod.get_axon_ntff_profile_hook = lambda: store[0]
    sys.modules["antenv.axon_hooks"] = mod

    lib = ctypes.CDLL("/opt/axon/libaxon_pjrt.so")
    if hasattr(lib, "axon_start_nrt_profile"):
        lib.axon_start_nrt_profile.argtypes = [
            ctypes.POINTER(ctypes.c_int64),
            ctypes.c_size_t,
        ]
        lib.axon_start_nrt_profile.restype = ctypes.c_int64
        lib.axon_stop_nrt_profile.argtypes = [ctypes.c_char_p]
        lib.axon_stop_nrt_profile.restype = ctypes.c_int64

        @contextlib.contextmanager
        def _hook(output_dir, device_ids):
            import jax

            jax.devices()
            if device_ids:
                ids = (ctypes.c_int64 * len(device_ids))(*device_ids)
                rc = lib.axon_start_nrt_profile(ids, len(device_ids))
            else:
                rc = lib.axon_start_nrt_profile(None, 0)
            if rc != 0:
                raise RuntimeError(f"axon_start_nrt_profile rc={rc}")
            try:
                yield
            finally:
                n = lib.axon_stop_nrt_profile(str(output_dir).encode())
                if n <= 0:
                    print(f"ntff profile wrote {n} files", flush=True)

        mod.set_axon_ntff_profile_hook(_hook)
    _CACHE["ntff"] = True


def kernel(x, expert_sel, w1, w2):
    y, _ = _run(x, expert_sel, w1, w2, trace=False)
    return y


# revision 26
# speedup vs baseline: 1.0101x; 1.0101x over previous
"""MoE (top-4 of 32 experts) Trainium2 kernel, data-parallel over 8 NeuronCores.

Problem shapes: x[4096,512] f32, expert_sel[32,512] f32, w1[32,512,128] f32,
w2[32,128,512] f32 -> y[4096,512] f32.

Strategy: shard the 4096 tokens 512-per-core (no collectives). Each core:
  - scores = x @ expert_sel.T in fp32 on PE (routing must match the
    reference's fp32 ranking exactly, so this stays fp32)
  - sigmoid on ACT; top-4 threshold per token via the DVE Max8 instruction;
    gate = sigmoid(score) * (score >= 4th_max), in bf16
  - gate rows are transposed to [E, T] with DVE 32x32 stream transposes,
    bounced through DRAM, and DMA-broadcast-read back as [128, T] per group
    of experts (SBUF APs cannot broadcast the partition dim; DRAM APs can)
  - dense expert MLPs in bf16, phase-split: all 32 experts' L1
    (h = relu(x @ w1[e]), gated to hg on DVE) first, then L2 in dc-major
    order (for each 128-wide output chunk: 32 accumulating matmuls into one
    PSUM bank) so each output chunk is evicted and DMA'd while the next
    chunk computes -- the write-out tail shrinks to one chunk.
  - PE warm-up starts immediately (DVE memset, not gpsimd) so the HAM clock
    gate reaches 8/8 during the input DMA stream.
  - all heavy DMAs are host-packed contiguous 2D transfers, enqueued on the
    SP HWDGE in need order.
"""

import os
import numpy as np
import ml_dtypes

N, D, E, H, K = 4096, 512, 32, 128, 4
NCORES = 8
TPC = N // NCORES  # tokens per core = 512
P = 128
DC = D // P        # 4 contraction chunks of d
TT = TPC // P      # 4 token tiles of 128
G = 4              # experts per DMA group
NG = E // G
BF16 = ml_dtypes.bfloat16

_CACHE = {}


def _split_waits_json(bir_bytes, max_waits=1):
    """The walrus build in this container encodes at most one sync-wait per
    instruction; Tile emits several on some. Split excess waits onto
    preceding same-engine NoOps (identical semantics: program order on the
    engine)."""
    import orjson

    bir = orjson.loads(bir_bytes)
    nid = [0]

    def fix_block(instructions):
        out = []
        for ins in instructions:
            si = ins.get("sync_info")
            waits = (si or {}).get("on_wait") or []
            if len(waits) > max_waits:
                chunks = [
                    waits[i : i + max_waits] for i in range(0, len(waits), max_waits)
                ]
                for ch in chunks[:-1]:
                    nid[0] += 1
                    out.append(
                        {
                            "opcode": "NoOp",
                            "engine": ins["engine"],
                            "ins": [],
                            "outs": [],
                            "name": f"I-ws-{nid[0]}",
                            "debug": ins.get("debug", 0),
                            "sync_info": {"on_wait": ch, "on_update": []},
                        }
                    )
                si["on_wait"] = chunks[-1]
            out.append(ins)
        return out

    def walk(o):
        if isinstance(o, dict):
            for k, v in o.items():
                if k == "instructions" and isinstance(v, list):
                    o[k] = fix_block(v)
                else:
                    walk(v)
        elif isinstance(o, list):
            for v in o:
                walk(v)

    walk(bir)
    return orjson.dumps(bir)


LDW_OPT = False


def _patch_compile():
    if _CACHE.get("patched"):
        return
    import concourse.bass2jax as bass2jax
    import concourse.bass_utils as bass_utils
    from concourse.bass_utils import compile_bir_kernel as _orig

    if LDW_OPT and not _CACHE.get("ldw_patched"):
        _orig_run = bass_utils.run_command

        def run2(cmd, **kw):
            cmd = [
                "--enable-ldw-opt=true" if c == "--enable-ldw-opt=false" else c
                for c in cmd
            ]
            return _orig_run(cmd, **kw)

        bass_utils.run_command = run2
        _CACHE["ldw_patched"] = True

    def patched(bir_json, tmpdir, neff_name="file.neff"):
        return _orig(_split_waits_json(bir_json), tmpdir, neff_name=neff_name)

    bass2jax.compile_bir_kernel = patched
    _CACHE["patched"] = True


def _patch_tile_tail():
    # Tile's kernel epilogue is drain + barrier + sem-clears + barrier. The
    # second barrier only orders the clears vs engine program-end; NRT
    # already requires every engine's program to finish before the NEFF can
    # run again, so it is dead time (~2-3us). Drop it.
    if _CACHE.get("tail_patched"):
        return
    from concourse.tile import TileContext
    from concourse.vector_clock import ScopedClock

    def _dab(self, tick_clock, wait_clock):
        drain_inst = self.nc.sync.drain()
        wait_clock.add_sem_waits(
            drain_inst.ins, ScopedClock({None: tick_clock.global_clock})
        )
        self.nc.all_engine_barrier()
        popped = self.nc._tile_sem_poison_stack.pop()
        assert popped is self._sem_poison
        self.nc.clear_and_free_semaphores(list(self.sems.allocated().values()))

    TileContext._drain_and_barrier = _dab
    _CACHE["tail_patched"] = True


def _build_nc():
    import concourse.bass as bass
    import concourse.mybir as mybir
    from concourse.tile import TileContext

    _patch_tile_tail()

    dt = mybir.dt
    Alu = mybir.AluOpType
    Act = mybir.ActivationFunctionType

    nc = bass.Bass()

    xT_d = nc.dram_tensor("xT", [P, DC * TPC], dt.float32, kind="ExternalInput")
    xbT_d = nc.dram_tensor("xbT", [P, DC * TPC], dt.bfloat16, kind="ExternalInput")
    selT_d = nc.dram_tensor("selT", [P, DC * E], dt.float32, kind="ExternalInput")
    # w1 packed per group of G experts: [NG, P(d-part), G * DC * H]
    w1_d = nc.dram_tensor("w1p", [NG, P, G * DC * H], dt.bfloat16, kind="ExternalInput")
    # w2 packed per group of G experts: [NG, P(h), G * D]
    w2_d = nc.dram_tensor("w2p", [NG, P, G * D], dt.bfloat16, kind="ExternalInput")
    yT_d = nc.dram_tensor("yT", [DC, P, TPC], dt.float32, kind="ExternalOutput")

    with TileContext(nc) as tc:
        with (
            tc.tile_pool(name="singles", bufs=1) as singles,
            tc.tile_pool(name="dram", bufs=1, space="DRAM") as drampool,
            tc.tile_pool(name="gpool", bufs=8) as gpool,
            tc.tile_pool(name="hrpool", bufs=20) as hrpool,
            tc.tile_pool(name="ph", bufs=4, space="PSUM") as phpool,
            tc.tile_pool(name="psc", bufs=2, space="PSUM") as pscpool,
            tc.tile_pool(name="py", bufs=2, space="PSUM") as pypool,
        ):
            xf = [
                singles.tile([P, DC * P], dt.float32, name=f"xf{tt}")
                for tt in range(TT)
            ]
            xb = singles.tile([P, DC * TPC], dt.bfloat16)
            sel = singles.tile([P, DC * E], dt.float32)
            sig = singles.tile([P, TT * E], dt.float32)
            m8 = singles.tile([P, TT * 8], dt.float32)
            gate = singles.tile([P, TT * E], dt.bfloat16)
            gTb = singles.tile([32, TPC], dt.bfloat16)
            w1s = singles.tile([P, E * DC * H], dt.bfloat16)   # 32KB/part
            w2s = singles.tile([P, E * D], dt.bfloat16)        # 32KB/part
            hgs = singles.tile([P, E * TPC], dt.bfloat16)      # 32KB/part
            y_sb = singles.tile([P, DC * TPC], dt.float32)
            gTd = drampool.tile([E, TPC], dt.bfloat16)

            ggrp = {}
            hrs = {}
            TH2 = TPC // 2

            def scores_tile(tt):
                psc = pscpool.tile([P, E], dt.float32, tag="psc", name=f"psc{tt}")
                for dc in range(DC):
                    nc.tensor.matmul(
                        psc[:],
                        xf[tt][:, dc * P : (dc + 1) * P],
                        sel[:, dc * E : (dc + 1) * E],
                        start=(dc == 0),
                        stop=(dc == DC - 1),
                    )
                sl = slice(tt * E, (tt + 1) * E)
                nc.scalar.activation(sig[:, sl], psc[:], Act.Sigmoid)
                # top-8 of the sigmoid (same ranking as raw scores)
                nc.vector.max(m8[:, tt * 8 : (tt + 1) * 8], sig[:, sl])
                # gate = (sig >= 4th max) * sig, in bf16
                nc.vector.scalar_tensor_tensor(
                    gate[:, sl],
                    sig[:, sl],
                    m8[:, tt * 8 + 3 : tt * 8 + 4],
                    sig[:, sl],
                    op0=Alu.is_ge,
                    op1=Alu.mult,
                )
                # transpose this token-tile of the gate to [E, T] and bounce
                # it through DRAM immediately (per-tile, to shorten the chain)
                for pb in range(TT):
                    nc.vector.transpose(
                        gTb[0:32, tt * P + pb * 32 : tt * P + (pb + 1) * 32],
                        gate[pb * 32 : (pb + 1) * 32, sl],
                    )
                if tt == TT - 1:
                    nc.gpsimd.dma_start(gTd[:], gTb[0:32, :])

            def g_group(gi):
                g = gpool.tile([P, G * TPC], dt.bfloat16, tag="g", name=f"g{gi}")
                base = gTd[gi * G : (gi + 1) * G, :]
                src = bass.AP(base.tensor, base.offset, [[0, P]] + list(base.ap))
                nc.sync.dma_start(g[:].rearrange("p (e t) -> p e t", e=G), src)
                ggrp[gi] = g

            def l1_mm(e):
                wbase = e * DC * H
                ph = phpool.tile([P, TPC], dt.float32, tag="ph", name=f"ph{e}")
                for dc in range(DC):
                    nc.tensor.matmul(
                        ph[:],
                        w1s[:, wbase + dc * H : wbase + (dc + 1) * H],
                        xb[:, dc * TPC : (dc + 1) * TPC],
                        start=(dc == 0),
                        stop=(dc == DC - 1),
                    )
                hr = hrpool.tile([P, TPC], dt.bfloat16, tag="hr", name=f"hr{e}")
                nc.scalar.activation(hr[:], ph[:], Act.Relu)
                hrs[e] = hr

            def l1_mul(e):
                gi, ei = divmod(e, G)
                hr = hrs.pop(e)
                nc.vector.tensor_mul(
                    hgs[:, e * TPC : (e + 1) * TPC],
                    hr[:],
                    ggrp[gi][:, ei * TPC : (ei + 1) * TPC],
                )

            def l2_chunk(dtile):
                py = pypool.tile([P, TPC], dt.float32, tag="py", name=f"py{dtile}")
                for e in range(E):
                    nc.tensor.matmul(
                        py[:],
                        w2s[:, e * D + dtile * P : e * D + (dtile + 1) * P],
                        hgs[:, e * TPC : (e + 1) * TPC],
                        start=(e == 0),
                        stop=(e == E - 1),
                    )
                sl = slice(dtile * TPC, (dtile + 1) * TPC)
                if dtile % 2 == 0:
                    nc.vector.tensor_copy(y_sb[:, sl], py[:])
                else:
                    nc.scalar.activation(y_sb[:, sl], py[:], Act.Copy)
                nc.sync.dma_start(yT_d[dtile], y_sb[:, sl])

            # SP HWDGE enqueue in need order. The routing inputs go first:
            # the gate path is the long pole.
            src = xT_d[:].rearrange("p (c t) -> p c t", c=DC)
            nc.sync.dma_start(
                xf[0][:].rearrange("p (c t) -> p c t", c=DC), src[:, :, :P]
            )
            nc.sync.dma_start(sel[:], selT_d[:])
            for tt in range(1, TT):
                nc.sync.dma_start(
                    xf[tt][:].rearrange("p (c t) -> p c t", c=DC),
                    src[:, :, tt * P : (tt + 1) * P],
                )
            nc.sync.dma_start(xb[:], xbT_d[:])
            for gi in range(NG):
                nc.sync.dma_start(
                    w1s[:, gi * G * DC * H : (gi + 1) * G * DC * H], w1_d[gi]
                )
            # PE keep-warm with dense 512-col matmuls while xb/w1g0 stream in
            # (small score matmuls hold the HAM clock down; dense ones ramp it)
            junk = singles.tile([P, TPC], dt.bfloat16)
            nc.vector.memset(junk[:], 1.0)
            pj = pscpool.tile([P, TPC], dt.float32, tag="psc", name="pjunk")
            for _ in range(6):
                nc.tensor.matmul(pj[:], junk[:, :P], junk[:], start=True, stop=True)
            # L1 starts immediately; scores slot in after 4 experts (the
            # deep hr pool buys the gate chain ~14us of slack). The gate
            # muls are emitted 4 experts late so the DVE runs the routing
            # chain before blocking on the first gate-broadcast read.
            for e in range(E):
                l1_mm(e)
                if e == 3:
                    for tt in range(TT):
                        scores_tile(tt)
                    for gi in range(NG):
                        g_group(gi)
                    for gi in range(NG):
                        nc.sync.dma_start(
                            w2s[:, gi * G * D : (gi + 1) * G * D], w2_d[gi]
                        )
                if e >= 4:
                    l1_mul(e - 4)
            for e in range(E - 4, E):
                l1_mul(e)
            for dtile in range(DC):
                l2_chunk(dtile)

    return nc


def _get_nc():
    if "nc" not in _CACHE:
        _CACHE["nc"] = _build_nc()
    return _CACHE["nc"]


def _pack_inputs(x, expert_sel, w1, w2):
    x = np.asarray(x, dtype=np.float32)
    expert_sel = np.asarray(expert_sel, dtype=np.float32)
    w1 = np.asarray(w1, dtype=np.float32)
    w2 = np.asarray(w2, dtype=np.float32)

    # selT: [p, dc*E + e] = expert_sel[e, dc*P + p]
    selT = np.ascontiguousarray(
        expert_sel.T.reshape(DC, P, E).transpose(1, 0, 2)
    ).reshape(P, DC * E)
    # w1 per expert: [p(d-part), dc*H + h], grouped by G experts
    w1p = (
        w1.astype(BF16).reshape(E, DC, P, H).transpose(0, 2, 1, 3).reshape(E, P, DC * H)
    )
    w1p = np.ascontiguousarray(
        w1p.reshape(NG, G, P, DC * H).transpose(0, 2, 1, 3)
    ).reshape(NG, P, G * DC * H)
    # w2 per expert: [p(h), d], grouped by G experts
    w2p = np.ascontiguousarray(
        w2.astype(BF16).reshape(NG, G, P, D).transpose(0, 2, 1, 3)
    ).reshape(NG, P, G * D)

    in_maps = []
    for c in range(NCORES):
        xc = x[c * TPC : (c + 1) * TPC]
        # xT: [p, dc*TPC + t] = x[t, dc*P + p]
        xT = np.ascontiguousarray(
            xc.T.reshape(DC, P, TPC).transpose(1, 0, 2)
        ).reshape(P, DC * TPC)
        in_maps.append(
            {"xT": xT, "xbT": xT.astype(BF16), "selT": selT, "w1p": w1p, "w2p": w2p}
        )
    return in_maps


def _run(x, expert_sel, w1, w2, trace=False, tmpdir=None):
    _patch_compile()
    from concourse.bass_utils import run_bass_kernel_spmd

    if trace:
        _install_ntff_hook()

    nc = _get_nc()
    in_maps = _pack_inputs(x, expert_sel, w1, w2)
    res = run_bass_kernel_spmd(
        nc, in_maps, list(range(NCORES)), trace=trace, tmpdir=tmpdir
    )
    y = np.empty((N, D), dtype=np.float32)
    for c in range(NCORES):
        yT = np.asarray(res.results[c]["yT"], dtype=np.float32)
        y[c * TPC : (c + 1) * TPC] = yT.reshape(D, TPC).T
    return y, res


def _install_ntff_hook():
    """Register the NTFF profiling hook (the container's antenv stub lacks
    axon_hooks; replicate trn_boot's ctypes hook). Also stub the artifact
    upload, which needs cloud storage not present here."""
    if _CACHE.get("ntff"):
        return
    import sys, types, ctypes, contextlib
    import antenv  # noqa: F401
    import concourse.bass_utils as bass_utils

    bass_utils.upload_artifacts = lambda d: f"file://{d}"

    mod = types.ModuleType("antenv.axon_hooks")
    store = [None]
    mod.set_axon_ntff_profile_hook = lambda h: store.__setitem__(0, h)
    mod.get_axon_ntff_profile_hook = lambda: store[0]
    sys.modules["antenv.axon_hooks"] = mod

    lib = ctypes.CDLL("/opt/axon/libaxon_pjrt.so")
    if hasattr(lib, "axon_start_nrt_profile"):
        lib.axon_start_nrt_profile.argtypes = [
            ctypes.POINTER(ctypes.c_int64),
            ctypes.c_size_t,
        ]
        lib.axon_start_nrt_profile.restype = ctypes.c_int64
        lib.axon_stop_nrt_profile.argtypes = [ctypes.c_char_p]
        lib.axon_stop_nrt_profile.restype = ctypes.c_int64

        @contextlib.contextmanager
        def _hook(output_dir, device_ids):
            import jax

            jax.devices()
            if device_ids:
                ids = (ctypes.c_int64 * len(device_ids))(*device_ids)
                rc = lib.axon_start_nrt_profile(ids, len(device_ids))
            else:
                rc = lib.axon_start_nrt_profile(None, 0)
            if rc != 0:
                raise RuntimeError(f"axon_start_nrt_profile rc={rc}")
            try:
                yield
            finally:
                n = lib.axon_stop_nrt_profile(str(output_dir).encode())
                if n <= 0:
                    print(f"ntff profile wrote {n} files", flush=True)

        mod.set_axon_ntff_profile_hook(_hook)
    _CACHE["ntff"] = True


def kernel(x, expert_sel, w1, w2):
    y, _ = _run(x, expert_sel, w1, w2, trace=False)
    return y


# revision 28
# speedup vs baseline: 1.0200x; 1.0098x over previous
"""MoE (top-4 of 32 experts) Trainium2 kernel, data-parallel over 8 NeuronCores.

Problem shapes: x[4096,512] f32, expert_sel[32,512] f32, w1[32,512,128] f32,
w2[32,128,512] f32 -> y[4096,512] f32.

Strategy: shard the 4096 tokens 512-per-core (no collectives). Each core:
  - scores = x @ expert_sel.T in fp32 on PE (routing must match the
    reference's fp32 ranking exactly, so this stays fp32)
  - sigmoid on ACT; top-4 threshold per token via the DVE Max8 instruction;
    gate = sigmoid(score) * (score >= 4th_max), in bf16
  - gate rows are transposed to [E, T] with DVE 32x32 stream transposes,
    bounced through DRAM, and DMA-broadcast-read back as [128, T] per group
    of experts (SBUF APs cannot broadcast the partition dim; DRAM APs can)
  - dense expert MLPs in bf16, phase-split: all 32 experts' L1
    (h = relu(x @ w1[e]), gated to hg on DVE) first, then L2 in dc-major
    order (for each 128-wide output chunk: 32 accumulating matmuls into one
    PSUM bank) so each output chunk is evicted and DMA'd while the next
    chunk computes -- the write-out tail shrinks to one chunk.
  - PE warm-up starts immediately (DVE memset, not gpsimd) so the HAM clock
    gate reaches 8/8 during the input DMA stream.
  - all heavy DMAs are host-packed contiguous 2D transfers, enqueued on the
    SP HWDGE in need order.
"""

import os
import numpy as np
import ml_dtypes

N, D, E, H, K = 4096, 512, 32, 128, 4
NCORES = 8
TPC = N // NCORES  # tokens per core = 512
P = 128
DC = D // P        # 4 contraction chunks of d
TT = TPC // P      # 4 token tiles of 128
G = 4              # experts per DMA group
NG = E // G
BF16 = ml_dtypes.bfloat16

_CACHE = {}


def _split_waits_json(bir_bytes, max_waits=1):
    """The walrus build in this container encodes at most one sync-wait per
    instruction; Tile emits several on some. Split excess waits onto
    preceding same-engine NoOps (identical semantics: program order on the
    engine)."""
    import orjson

    bir = orjson.loads(bir_bytes)
    nid = [0]

    def fix_block(instructions):
        out = []
        for ins in instructions:
            si = ins.get("sync_info")
            waits = (si or {}).get("on_wait") or []
            if len(waits) > max_waits:
                chunks = [
                    waits[i : i + max_waits] for i in range(0, len(waits), max_waits)
                ]
                for ch in chunks[:-1]:
                    nid[0] += 1
                    out.append(
                        {
                            "opcode": "NoOp",
                            "engine": ins["engine"],
                            "ins": [],
                            "outs": [],
                            "name": f"I-ws-{nid[0]}",
                            "debug": ins.get("debug", 0),
                            "sync_info": {"on_wait": ch, "on_update": []},
                        }
                    )
                si["on_wait"] = chunks[-1]
            out.append(ins)
        return out

    def walk(o):
        if isinstance(o, dict):
            for k, v in o.items():
                if k == "instructions" and isinstance(v, list):
                    o[k] = fix_block(v)
                else:
                    walk(v)
        elif isinstance(o, list):
            for v in o:
                walk(v)

    walk(bir)
    return orjson.dumps(bir)


LDW_OPT = False


def _patch_compile():
    if _CACHE.get("patched"):
        return
    import concourse.bass2jax as bass2jax
    import concourse.bass_utils as bass_utils
    from concourse.bass_utils import compile_bir_kernel as _orig

    if LDW_OPT and not _CACHE.get("ldw_patched"):
        _orig_run = bass_utils.run_command

        def run2(cmd, **kw):
            cmd = [
                "--enable-ldw-opt=true" if c == "--enable-ldw-opt=false" else c
                for c in cmd
            ]
            return _orig_run(cmd, **kw)

        bass_utils.run_command = run2
        _CACHE["ldw_patched"] = True

    def patched(bir_json, tmpdir, neff_name="file.neff"):
        return _orig(_split_waits_json(bir_json), tmpdir, neff_name=neff_name)

    bass2jax.compile_bir_kernel = patched
    _CACHE["patched"] = True


def _patch_tile_tail():
    # Tile's kernel epilogue is drain + barrier + sem-clears + barrier. The
    # second barrier only orders the clears vs engine program-end; NRT
    # already requires every engine's program to finish before the NEFF can
    # run again, so it is dead time (~2-3us). Drop it.
    if _CACHE.get("tail_patched"):
        return
    from concourse.tile import TileContext
    from concourse.vector_clock import ScopedClock

    def _dab(self, tick_clock, wait_clock):
        drain_inst = self.nc.sync.drain()
        wait_clock.add_sem_waits(
            drain_inst.ins, ScopedClock({None: tick_clock.global_clock})
        )
        self.nc.all_engine_barrier()
        popped = self.nc._tile_sem_poison_stack.pop()
        assert popped is self._sem_poison
        self.nc.clear_and_free_semaphores(list(self.sems.allocated().values()))

    TileContext._drain_and_barrier = _dab
    _CACHE["tail_patched"] = True


def _build_nc():
    import concourse.bass as bass
    import concourse.mybir as mybir
    from concourse.tile import TileContext

    _patch_tile_tail()

    dt = mybir.dt
    Alu = mybir.AluOpType
    Act = mybir.ActivationFunctionType

    nc = bass.Bass()

    xT_d = nc.dram_tensor("xT", [P, DC * TPC], dt.float32, kind="ExternalInput")
    xbT_d = nc.dram_tensor("xbT", [P, DC * TPC], dt.bfloat16, kind="ExternalInput")
    selT_d = nc.dram_tensor("selT", [P, DC * E], dt.float32, kind="ExternalInput")
    # w1 packed per group of G experts: [NG, P(d-part), G * DC * H]
    w1_d = nc.dram_tensor("w1p", [NG, P, G * DC * H], dt.bfloat16, kind="ExternalInput")
    # w2 packed per group of G experts: [NG, P(h), G * D]
    w2_d = nc.dram_tensor("w2p", [NG, P, G * D], dt.bfloat16, kind="ExternalInput")
    yT_d = nc.dram_tensor("yT", [DC, P, TPC], dt.float32, kind="ExternalOutput")

    with TileContext(nc) as tc:
        with (
            tc.tile_pool(name="singles", bufs=1) as singles,
            tc.tile_pool(name="dram", bufs=1, space="DRAM") as drampool,
            tc.tile_pool(name="gpool", bufs=8) as gpool,
            tc.tile_pool(name="hrpool", bufs=20) as hrpool,
            tc.tile_pool(name="ph", bufs=4, space="PSUM") as phpool,
            tc.tile_pool(name="psc", bufs=2, space="PSUM") as pscpool,
            tc.tile_pool(name="py", bufs=2, space="PSUM") as pypool,
        ):
            xf = [
                singles.tile([P, DC * P], dt.float32, name=f"xf{tt}")
                for tt in range(TT)
            ]
            xb = singles.tile([P, DC * TPC], dt.bfloat16)
            sel = singles.tile([P, DC * E], dt.float32)
            sig = singles.tile([P, TT * E], dt.float32)
            m8 = singles.tile([P, TT * 8], dt.float32)
            gate = singles.tile([P, TT * E], dt.bfloat16)
            gTb = singles.tile([32, TPC], dt.bfloat16)
            w1s = singles.tile([P, E * DC * H], dt.bfloat16)   # 32KB/part
            w2s = singles.tile([P, E * D], dt.bfloat16)        # 32KB/part
            hgs = singles.tile([P, E * TPC], dt.bfloat16)      # 32KB/part
            y_sb = singles.tile([P, DC * TPC], dt.float32)
            gTd = drampool.tile([E, TPC], dt.bfloat16)

            ggrp = {}
            hrs = {}
            TH2 = TPC // 2

            def scores_tile(tt):
                psc = pscpool.tile([P, E], dt.float32, tag="psc", name=f"psc{tt}")
                for dc in range(DC):
                    nc.tensor.matmul(
                        psc[:],
                        xf[tt][:, dc * P : (dc + 1) * P],
                        sel[:, dc * E : (dc + 1) * E],
                        start=(dc == 0),
                        stop=(dc == DC - 1),
                    )
                sl = slice(tt * E, (tt + 1) * E)
                nc.scalar.activation(sig[:, sl], psc[:], Act.Sigmoid)
                # top-8 of the sigmoid (same ranking as raw scores)
                nc.vector.max(m8[:, tt * 8 : (tt + 1) * 8], sig[:, sl])
                # gate = (sig >= 4th max) * sig, in bf16
                nc.vector.scalar_tensor_tensor(
                    gate[:, sl],
                    sig[:, sl],
                    m8[:, tt * 8 + 3 : tt * 8 + 4],
                    sig[:, sl],
                    op0=Alu.is_ge,
                    op1=Alu.mult,
                )
                # transpose this token-tile of the gate to [E, T] and bounce
                # it through DRAM immediately (per-tile, to shorten the chain)
                for pb in range(TT):
                    nc.vector.transpose(
                        gTb[0:32, tt * P + pb * 32 : tt * P + (pb + 1) * 32],
                        gate[pb * 32 : (pb + 1) * 32, sl],
                    )
                if tt == TT - 1:
                    nc.gpsimd.dma_start(gTd[:], gTb[0:32, :])

            def g_group(gi):
                g = gpool.tile([P, G * TPC], dt.bfloat16, tag="g", name=f"g{gi}")
                base = gTd[gi * G : (gi + 1) * G, :]
                src = bass.AP(base.tensor, base.offset, [[0, P]] + list(base.ap))
                nc.sync.dma_start(g[:].rearrange("p (e t) -> p e t", e=G), src)
                ggrp[gi] = g

            def l1_mm(e):
                wbase = e * DC * H
                ph = phpool.tile([P, TPC], dt.float32, tag="ph", name=f"ph{e}")
                for dc in range(DC):
                    nc.tensor.matmul(
                        ph[:],
                        w1s[:, wbase + dc * H : wbase + (dc + 1) * H],
                        xb[:, dc * TPC : (dc + 1) * TPC],
                        start=(dc == 0),
                        stop=(dc == DC - 1),
                    )
                hr = hrpool.tile([P, TPC], dt.bfloat16, tag="hr", name=f"hr{e}")
                nc.scalar.activation(hr[:], ph[:], Act.Relu)
                hrs[e] = hr

            def l1_mul(e):
                gi, ei = divmod(e, G)
                hr = hrs.pop(e)
                nc.vector.tensor_mul(
                    hgs[:, e * TPC : (e + 1) * TPC],
                    hr[:],
                    ggrp[gi][:, ei * TPC : (ei + 1) * TPC],
                )

            def l2_chunk(dtile):
                py = pypool.tile([P, TPC], dt.float32, tag="py", name=f"py{dtile}")
                for e in range(E):
                    nc.tensor.matmul(
                        py[:],
                        w2s[:, e * D + dtile * P : e * D + (dtile + 1) * P],
                        hgs[:, e * TPC : (e + 1) * TPC],
                        start=(e == 0),
                        stop=(e == E - 1),
                    )
                sl = slice(dtile * TPC, (dtile + 1) * TPC)
                if dtile % 2 == 0:
                    nc.vector.tensor_copy(y_sb[:, sl], py[:])
                else:
                    nc.scalar.activation(y_sb[:, sl], py[:], Act.Copy)
                nc.sync.dma_start(yT_d[dtile], y_sb[:, sl])

            # SP HWDGE enqueue in need order: L1's inputs first (xb + w1g0),
            # then the score inputs (scores run later, after the clock ramp),
            # then the rest of the weights.
            src = xT_d[:].rearrange("p (c t) -> p c t", c=DC)
            nc.sync.dma_start(xb[:], xbT_d[:])
            nc.sync.dma_start(w1s[:, : G * DC * H], w1_d[0])
            for tt in range(TT):
                nc.sync.dma_start(
                    xf[tt][:].rearrange("p (c t) -> p c t", c=DC),
                    src[:, :, tt * P : (tt + 1) * P],
                )
            nc.sync.dma_start(sel[:], selT_d[:])
            for gi in range(1, NG):
                nc.sync.dma_start(
                    w1s[:, gi * G * DC * H : (gi + 1) * G * DC * H], w1_d[gi]
                )
            # PE keep-warm with dense 512-col matmuls while xb/w1g0 stream in
            # (small score matmuls hold the HAM clock down; dense ones ramp it)
            junk = singles.tile([P, TPC], dt.bfloat16)
            nc.vector.memset(junk[:], 1.0)
            pj = pscpool.tile([P, TPC], dt.float32, tag="psc", name="pjunk")
            for _ in range(5):
                nc.tensor.matmul(pj[:], junk[:, :P], junk[:], start=True, stop=True)
            # L1 starts immediately; scores slot in after 8 experts -- by
            # then the HAM clock is at 8/8 so the fp32 score matmuls cost
            # half as much, and the deep hr pool buys the gate chain the
            # slack. Gate muls are emitted late so the DVE runs the routing
            # chain before blocking on the first gate-broadcast read.
            SCORE_AT = 8
            for e in range(E):
                l1_mm(e)
                if e == SCORE_AT:
                    for tt in range(TT):
                        scores_tile(tt)
                    for gi in range(NG):
                        g_group(gi)
                    for gi in range(NG):
                        nc.sync.dma_start(
                            w2s[:, gi * G * D : (gi + 1) * G * D], w2_d[gi]
                        )
                if e > SCORE_AT:
                    l1_mul(e - SCORE_AT - 1)
            for e in range(E - SCORE_AT - 1, E):
                l1_mul(e)
            for dtile in range(DC):
                l2_chunk(dtile)

    return nc


def _get_nc():
    if "nc" not in _CACHE:
        _CACHE["nc"] = _build_nc()
    return _CACHE["nc"]


def _pack_inputs(x, expert_sel, w1, w2):
    x = np.asarray(x, dtype=np.float32)
    expert_sel = np.asarray(expert_sel, dtype=np.float32)
    w1 = np.asarray(w1, dtype=np.float32)
    w2 = np.asarray(w2, dtype=np.float32)

    # selT: [p, dc*E + e] = expert_sel[e, dc*P + p]
    selT = np.ascontiguousarray(
        expert_sel.T.reshape(DC, P, E).transpose(1, 0, 2)
    ).reshape(P, DC * E)
    # w1 per expert: [p(d-part), dc*H + h], grouped by G experts
    w1p = (
        w1.astype(BF16).reshape(E, DC, P, H).transpose(0, 2, 1, 3).reshape(E, P, DC * H)
    )
    w1p = np.ascontiguousarray(
        w1p.reshape(NG, G, P, DC * H).transpose(0, 2, 1, 3)
    ).reshape(NG, P, G * DC * H)
    # w2 per expert: [p(h), d], grouped by G experts
    w2p = np.ascontiguousarray(
        w2.astype(BF16).reshape(NG, G, P, D).transpose(0, 2, 1, 3)
    ).reshape(NG, P, G * D)

    in_maps = []
    for c in range(NCORES):
        xc = x[c * TPC : (c + 1) * TPC]
        # xT: [p, dc*TPC + t] = x[t, dc*P + p]
        xT = np.ascontiguousarray(
            xc.T.reshape(DC, P, TPC).transpose(1, 0, 2)
        ).reshape(P, DC * TPC)
        in_maps.append(
            {"xT": xT, "xbT": xT.astype(BF16), "selT": selT, "w1p": w1p, "w2p": w2p}
        )
    return in_maps


def _run(x, expert_sel, w1, w2, trace=False, tmpdir=None):
    _patch_compile()
    from concourse.bass_utils import run_bass_kernel_spmd

    if trace:
        _install_ntff_hook()

    nc = _get_nc()
    in_maps = _pack_inputs(x, expert_sel, w1, w2)
    res = run_bass_kernel_spmd(
        nc, in_maps, list(range(NCORES)), trace=trace, tmpdir=tmpdir
    )
    y = np.empty((N, D), dtype=np.float32)
    for c in range(NCORES):
        yT = np.asarray(res.results[c]["yT"], dtype=np.float32)
        y[c * TPC : (c + 1) * TPC] = yT.reshape(D, TPC).T
    return y, res


def _install_ntff_hook():
    """Register the NTFF profiling hook (the container's antenv stub lacks
    axon_hooks; replicate trn_boot's ctypes hook). Also stub the artifact
    upload, which needs cloud storage not present here."""
    if _CACHE.get("ntff"):
        return
    import sys, types, ctypes, contextlib
    import antenv  # noqa: F401
    import concourse.bass_utils as bass_utils

    bass_utils.upload_artifacts = lambda d: f"file://{d}"

    mod = types.ModuleType("antenv.axon_hooks")
    store = [None]
    mod.set_axon_ntff_profile_hook = lambda h: store.__setitem__(0, h)
    mod.get_axon_ntff_profile_hook = lambda: store[0]
    sys.modules["antenv.axon_hooks"] = mod

    lib = ctypes.CDLL("/opt/axon/libaxon_pjrt.so")
    if hasattr(lib, "axon_start_nrt_profile"):
        lib.axon_start_nrt_profile.argtypes = [
            ctypes.POINTER(ctypes.c_int64),
            ctypes.c_size_t,
        ]
        lib.axon_start_nrt_profile.restype = ctypes.c_int64
        lib.axon_stop_nrt_profile.argtypes = [ctypes.c_char_p]
        lib.axon_stop_nrt_profile.restype = ctypes.c_int64

        @contextlib.contextmanager
        def _hook(output_dir, device_ids):
            import jax

            jax.devices()
            if device_ids:
                ids = (ctypes.c_int64 * len(device_ids))(*device_ids)
                rc = lib.axon_start_nrt_profile(ids, len(device_ids))
            else:
                rc = lib.axon_start_nrt_profile(None, 0)
            if rc != 0:
                raise RuntimeError(f"axon_start_nrt_profile rc={rc}")
            try:
                yield
            finally:
                n = lib.axon_stop_nrt_profile(str(output_dir).encode())
                if n <= 0:
                    print(f"ntff profile wrote {n} files", flush=True)

        mod.set_axon_ntff_profile_hook(_hook)
    _CACHE["ntff"] = True


def kernel(x, expert_sel, w1, w2):
    y, _ = _run(x, expert_sel, w1, w2, trace=False)
    return y
